# revision 1
# baseline (speedup 1.0000x reference)
"""Trainium2 Bass kernel for nn_Attention_43542378447097.

GroupNorm -> multi-head causal self-attention -> out-proj, then the
reference's broadcast add:

    out(B,S,C) + residual(B,C,1,C)  ->  (B,C,S,C)   [right-aligned numpy
    broadcasting, so batches MIX]:

    result[i, j, k, l] = A[j, k, l] + xn[i, j, l]

where A[j] = attention output (incl bo) of batch j and xn[i] = groupnorm
output of batch i.  Output is (96, 96, 96, 96) fp32 (~340 MB) -> memory
bound; ~42.5 MB written per core.

Sharding: core c owns batches/rows i in [12c, 12c+12).
  Phase 1 (per local batch): groupnorm + attention -> A_local (12,96,96)
  Phase 2: AllGather A_local over 8 cores -> A_full (96,96,96), ~3.5 MB
  Phase 3 (per local i): result[i] = A_full + (xn_i + bo_eff) broadcast
    over k -- elementwise adds with a stride-0 middle-dim broadcast on
    in1, emitted as 24 half-slabs interleaved between VectorE (16) and
    GpSimd (8) so both streams run concurrently against the output DMA
    (one 1.77 MB DMA per half-slab).

Attention layout choices avoid all cross-partition broadcasts:
  qT/kT per head via lhsT=W-slice, rhs=xnT;  v natural via lhsT=xnT.
  scoresT = kT_h.T @ qT_h  ->  exp on ACT -> causal mask multiply (one
  DVE op over all heads) -> softmax denominators via ones-matmul (sums
  over partitions, result replicated across partitions) -> reciprocal ->
  attnT -> oT_h = v_h.T @ attnT_h -> out = sum_h ocatT_h.T @ Wo_h.
1/sqrt(dk) folded into Wq/bq on host; q/k biases folded into the matmuls
as a 97th contraction row; bv folded into bo_eff = bv@Wo+bo (softmax rows
sum to 1); groupnorm rstd is an all-DVE Newton rsqrt so the ACT engine
only ever loads the Exp table set (one table load total).
"""

import sys

sys.path.insert(0, "/opt/trn_rl_repo")

import numpy as np

B_TOTAL = 96
C = 96
S = 96
NH = 8
DK = 96
G = 8
NCORES = 8
BPC = B_TOTAL // NCORES  # 12
EPS = 1e-5
NFREE = S * C  # 9216
HALFN = NFREE // 2  # assembly half-slab width
# assembly half-slabs 0..23 interleaved between VectorE (16) and GpSimd (8):
# GpSimd is ~2x slower per element and also runs the phase-1 causal masks.
_GPSIMD_HALVES = frozenset(range(1, 24, 3))

_PROG = None


def _build_program(skip_collective=False, loop_n=1, phases="123"):
    import contextlib

    import concourse.bass as bass
    import concourse.tile as tile
    from concourse import bacc, mybir

    f32 = mybir.dt.float32
    AF = mybir.ActivationFunctionType
    ALU = mybir.AluOpType
    AX = mybir.AxisListType

    nc = bacc.Bacc(
        "TRN2",
        target_bir_lowering=False,
        debug=False,
        enable_asserts=False,
        num_devices=NCORES,
    )

    x_d = nc.declare_dram_parameter("x", [BPC, C, C], f32, isOutput=False)
    # wq/wk carry the bias as a 97th contraction row (paired with a ones row
    # appended to xnT), so q/k evictions are plain copies.
    wq_d = nc.declare_dram_parameter("wq", [C + 1, NH, DK], f32, isOutput=False)
    wk_d = nc.declare_dram_parameter("wk", [C + 1, NH, DK], f32, isOutput=False)
    wv_d = nc.declare_dram_parameter("wv", [C, NH, DK], f32, isOutput=False)
    wo_d = nc.declare_dram_parameter("wo", [DK, NH, C], f32, isOutput=False)
    gamma_d = nc.declare_dram_parameter("gamma", [C, 1], f32, isOutput=False)
    beta_d = nc.declare_dram_parameter("beta", [C, 1], f32, isOutput=False)
    gmask_d = nc.declare_dram_parameter("gmask", [C, C], f32, isOutput=False)
    ones_d = nc.declare_dram_parameter("ones96", [S, S], f32, isOutput=False)
    maskt_d = nc.declare_dram_parameter("maskT", [S, S], f32, isOutput=False)
    iden_d = nc.declare_dram_parameter("iden", [C, C], f32, isOutput=False)
    boe_d = nc.declare_dram_parameter("bo_eff", [1, C], f32, isOutput=False)
    out_d = nc.declare_dram_parameter("out", [BPC, C, NFREE], f32, isOutput=True)

    with tile.TileContext(nc) as tc:
        with (
            tc.tile_pool(name="const", bufs=1) as cpool,
            tc.tile_pool(name="work", bufs=2) as work,
            tc.tile_pool(name="psum", bufs=6, space="PSUM") as pp,
            tc.tile_pool(name="dram", bufs=1, space="DRAM") as dpool,
        ):
            # ---- constants ----
            wq_sb = cpool.tile([C + 1, NH, DK], f32, name="wq_sb")
            wk_sb = cpool.tile([C + 1, NH, DK], f32, name="wk_sb")
            wv_sb = cpool.tile([C, NH, DK], f32, name="wv_sb")
            wo_sb = cpool.tile([DK, NH, C], f32, name="wo_sb")
            gamma_sb = cpool.tile([C, 1], f32, name="gamma_sb")
            beta_sb = cpool.tile([C, 1], f32, name="beta_sb")
            gmask_sb = cpool.tile([C, C], f32, name="gmask_sb")
            ones_sb = cpool.tile([S, S], f32, name="ones_sb")
            maskt_sb = cpool.tile([S, S], f32, name="maskt_sb")
            iden_sb = cpool.tile([C, C], f32, name="iden_sb")
            eps_sb = cpool.tile([C, 1], f32, name="eps_sb")
            bo_rep = cpool.tile([C, C], f32, name="bo_rep")
            xnp_all = cpool.tile([C, BPC, C], f32, name="xnp_all")
            a_sb = cpool.tile([C, NFREE], f32, name="a_sb")

            nc.sync.dma_start(out=wq_sb, in_=wq_d[:])
            nc.sync.dma_start(out=wk_sb, in_=wk_d[:])
            nc.sync.dma_start(out=wv_sb, in_=wv_d[:])
            nc.sync.dma_start(out=wo_sb, in_=wo_d[:])
            nc.sync.dma_start(out=gamma_sb, in_=gamma_d[:])
            nc.sync.dma_start(out=beta_sb, in_=beta_d[:])
            nc.sync.dma_start(out=gmask_sb, in_=gmask_d[:])
            nc.sync.dma_start(out=ones_sb, in_=ones_d[:])
            nc.sync.dma_start(out=maskt_sb, in_=maskt_d[:])
            nc.sync.dma_start(out=iden_sb, in_=iden_d[:])
            nc.sync.dma_start(out=bo_rep, in_=boe_d[:].to_broadcast((C, C)))
            nc.vector.memset(eps_sb, EPS)

            # DRAM bounce buffers for the collective
            a_loc = dpool.tile([BPC, S, C], f32, name="a_loc")
            a_full = dpool.tile(
                [NCORES * BPC, S, C],
                f32,
                name="a_full",
                addr_space="Local" if skip_collective else "Shared",
            )

            inv_n = 1.0 / (C * C // G)  # 1/1152

            loop_cm = (
                tc.For_i(0, loop_n, 1)
                if loop_n > 1
                else contextlib.nullcontext()
            )
            loop_cm.__enter__()

            # ===== phase 1: local groupnorm + attention, 5-stage software
            # pipeline: emission interleaves adjacent batches so each
            # engine's in-order stream always has independent work and
            # cross-engine hop latencies are hidden.
            st = {}

            def st1(b):
                d = st[b] = {}
                x_sb = work.tile([C, C], f32, tag="x_sb", bufs=3, name="x_sb")
                nc.sync.dma_start(out=x_sb, in_=x_d[b])
                x2_sb = work.tile([C, C], f32, tag="x2_sb", name="x2_sb")
                nc.vector.tensor_mul(x2_sb, x_sb, x_sb)
                ps1 = pp.tile([C, C], f32, tag="ps", name="ps_s1")
                nc.tensor.matmul(ps1, lhsT=gmask_sb, rhs=x_sb, start=True, stop=True)
                ps2 = pp.tile([C, C], f32, tag="ps", name="ps_s2")
                nc.tensor.matmul(ps2, lhsT=gmask_sb, rhs=x2_sb, start=True, stop=True)
                s1r = work.tile([C, 1], f32, tag="st", bufs=8, name="s1r")
                s2r = work.tile([C, 1], f32, tag="st", bufs=8, name="s2r")
                nc.vector.tensor_reduce(out=s1r, in_=ps1, axis=AX.X, op=ALU.add)
                nc.vector.tensor_reduce(out=s2r, in_=ps2, axis=AX.X, op=ALU.add)
                mu = work.tile([C, 1], f32, tag="st", bufs=8, name="mu")
                ex2 = work.tile([C, 1], f32, tag="st", bufs=8, name="ex2")
                nc.vector.tensor_scalar_mul(mu, s1r, inv_n)
                nc.vector.tensor_scalar_mul(ex2, s2r, inv_n)
                musq = work.tile([C, 1], f32, tag="st", bufs=8, name="musq")
                nc.vector.tensor_mul(musq, mu, mu)
                veps = work.tile([C, 1], f32, tag="st", bufs=8, name="veps")
                nc.vector.scalar_tensor_tensor(
                    veps, ex2, EPS, musq, op0=ALU.add, op1=ALU.subtract
                )
                # rstd = rsqrt(veps), all-DVE (quake seed + 2 Newton steps) so
                # ACT only ever needs the Exp table set.
                i32 = mybir.dt.int32
                iv = veps.bitcast(i32)
                ineg = work.tile([C, 1], i32, tag="sti", bufs=8, name="ineg")
                nc.vector.tensor_scalar_mul(ineg, iv, -1)
                nc.vector.tensor_scalar(ineg, ineg, 1, None, op0=ALU.arith_shift_right)
                nc.vector.tensor_scalar(ineg, ineg, 0x5F3759DF, None, op0=ALU.add)
                y = ineg.bitcast(f32)
                t1 = work.tile([C, 1], f32, tag="st", bufs=8, name="t1")
                for _ in range(2):
                    nc.vector.tensor_mul(t1, y, y)
                    nc.vector.tensor_mul(t1, t1, veps)
                    nc.vector.tensor_scalar(t1, t1, -0.5, 1.5, op0=ALU.mult, op1=ALU.add)
                    nc.vector.tensor_mul(y, y, t1)
                scale_t = work.tile([C, 1], f32, tag="st", bufs=8, name="scale_t")
                nc.vector.tensor_mul(scale_t, y, gamma_sb)
                mus = work.tile([C, 1], f32, tag="st", bufs=8, name="mus")
                nc.vector.tensor_mul(mus, mu, scale_t)
                shift_t = work.tile([C, 1], f32, tag="st", bufs=8, name="shift_t")
                nc.vector.tensor_sub(shift_t, beta_sb, mus)
                xn_sb = work.tile([C, C], f32, tag="xn_sb", name="xn_sb")
                nc.vector.tensor_scalar(
                    xn_sb, x_sb, scale_t, shift_t, op0=ALU.mult, op1=ALU.add
                )
                nc.vector.tensor_add(xnp_all[:, b, :], xn_sb, bo_rep)
                ps_xt = pp.tile([C, C], f32, tag="ps", name="ps_xt")
                nc.tensor.transpose(ps_xt, xn_sb, iden_sb)
                xnT = work.tile([C + 1, C], f32, tag="xnT", bufs=4, name="xnT")
                nc.any.tensor_copy(out=xnT[0:C, :], in_=ps_xt)
                nc.vector.memset(xnT[C : C + 1, :], 1.0)
                d["xnT"] = xnT

            def st2(b):
                d = st[b]
                xnT = d["xnT"]
                qT_sb = work.tile([DK, NH, S], f32, tag="qT_sb", bufs=4, name="qT_sb")
                kT_sb = work.tile([DK, NH, S], f32, tag="kT_sb", bufs=4, name="kT_sb")
                v_sb = work.tile([S, NH, DK], f32, tag="v_sb", bufs=4, name="v_sb")
                for h in range(NH):
                    psq = pp.tile([DK, S], f32, tag="ps", name="ps_q")
                    nc.tensor.matmul(
                        psq, lhsT=wq_sb[:, h, :], rhs=xnT, start=True, stop=True
                    )
                    nc.any.tensor_copy(out=qT_sb[:, h, :], in_=psq)
                    psk = pp.tile([DK, S], f32, tag="ps", name="ps_k")
                    nc.tensor.matmul(
                        psk, lhsT=wk_sb[:, h, :], rhs=xnT, start=True, stop=True
                    )
                    nc.any.tensor_copy(out=kT_sb[:, h, :], in_=psk)
                    psv = pp.tile([S, DK], f32, tag="ps", name="ps_v")
                    nc.tensor.matmul(
                        psv, lhsT=xnT[0:C, :], rhs=wv_sb[:, h, :], start=True, stop=True
                    )
                    nc.any.tensor_copy(out=v_sb[:, h, :], in_=psv)
                d["qT"], d["kT"], d["v"] = qT_sb, kT_sb, v_sb

            def st3(b):
                d = st[b]
                expT_sb = work.tile([S, NH, S], f32, tag="expT_sb", bufs=4, name="expT_sb")
                for h in range(NH):
                    pst = pp.tile([S, S], f32, tag="ps", name="ps_sc")
                    nc.tensor.matmul(
                        pst,
                        lhsT=d["kT"][:, h, :],
                        rhs=d["qT"][:, h, :],
                        start=True,
                        stop=True,
                    )
                    nc.scalar.activation(out=expT_sb[:, h, :], in_=pst, func=AF.Exp)
                    nc.vector.tensor_mul(expT_sb[:, h, :], expT_sb[:, h, :], maskt_sb)
                d["expT"] = expT_sb

            def st4(b):
                d = st[b]
                expT_sb = d["expT"]
                recip_sb = work.tile([S, NH * S], f32, tag="recip_sb", name="recip_sb")
                for hh in range(2):
                    psd = pp.tile([S, 4 * S], f32, tag="ps", name="ps_den")
                    nc.tensor.matmul(
                        psd,
                        lhsT=ones_sb,
                        rhs=expT_sb[:, 4 * hh : 4 * (hh + 1), :].rearrange(
                            "p h s -> p (h s)"
                        ),
                        start=True,
                        stop=True,
                    )
                    nc.vector.reciprocal(
                        out=recip_sb[:, hh * 4 * S : (hh + 1) * 4 * S], in_=psd
                    )
                nc.vector.tensor_mul(
                    expT_sb, expT_sb, recip_sb.rearrange("p (h s) -> p h s", h=NH)
                )

            def st5(b):
                d = st.pop(b)
                ocatT_sb = work.tile([DK, NH, S], f32, tag="ocatT_sb", name="ocatT_sb")
                for h in range(NH):
                    pso = pp.tile([DK, S], f32, tag="ps", name="ps_o")
                    nc.tensor.matmul(
                        pso,
                        lhsT=d["v"][:, h, :],
                        rhs=d["expT"][:, h, :],
                        start=True,
                        stop=True,
                    )
                    nc.any.tensor_copy(out=ocatT_sb[:, h, :], in_=pso)
                psw = pp.tile([S, C], f32, tag="psw", bufs=2, name="ps_w")
                for h in range(NH):
                    nc.tensor.matmul(
                        psw,
                        lhsT=ocatT_sb[:, h, :],
                        rhs=wo_sb[:, h, :],
                        start=(h == 0),
                        stop=(h == NH - 1),
                    )
                outp_sb = work.tile([S, C], f32, tag="outp_sb", name="outp_sb")
                nc.any.tensor_copy(out=outp_sb, in_=psw)
                nc.sync.dma_start(out=a_loc[b], in_=outp_sb)

            # Pair-interleaved emission: two batches advance stage-by-stage
            # together, so every engine's in-order stream alternates between
            # independent batches (hiding cross-engine hop latency) while
            # only two batches compete for PSUM slots. A deeper 5-stage skew
            # measured slower on HW (PSUM slot contention); fully sequential
            # emission leaves each engine stalled on the serial chain.
            if "1" in phases:
                for b0 in range(0, BPC, 3):
                    for fn in (st1, st2, st3, st4, st5):
                        fn(b0)
                        fn(b0 + 1)
                        fn(b0 + 2)

            # ================= phase 2: all-gather attention outputs =======
            if "2" not in phases:
                pass
            elif skip_collective:
                # timeline-sim variant: approximate the collective's DMA cost
                for cc in range(NCORES):
                    nc.sync.dma_start(
                        out=a_full[cc * BPC : (cc + 1) * BPC], in_=a_loc[:]
                    )
            else:
                nc.gpsimd.collective_compute(
                    "AllGather",
                    mybir.AluOpType.bypass,
                    replica_groups=[list(range(NCORES))],
                    ins=[a_loc.opt()],
                    outs=[a_full.opt()],
                )
            if "2" in phases:
                # load in k-halves so half-0 assembly overlaps the second DMA
                a_flat = a_full[:].rearrange("j k l -> j (k l)")
                nc.sync.dma_start(
                    out=a_sb[:, 0:HALFN], in_=a_flat[:, 0:HALFN]
                )
                nc.sync.dma_start(
                    out=a_sb[:, HALFN:NFREE], in_=a_flat[:, HALFN:NFREE]
                )
            a_3d = a_sb.rearrange("p (k l) -> p k l", l=C)

            # ================= phase 3: assemble + write output ============
            # half-slabs interleaved between DVE and GpSimd so both engine
            # streams run concurrently against the output DMA.
            KH = S // 2  # 48 k-rows per half-slab
            for i in range(BPC) if "3" in phases else []:
                for half in range(2):
                    g = i * 2 + half
                    res_t = work.tile([C, HALFN], f32, tag="res", bufs=3)
                    eng = nc.gpsimd if g in _GPSIMD_HALVES else nc.vector
                    eng.tensor_tensor(
                        res_t.rearrange("p (k l) -> p k l", l=C),
                        a_3d[:, half * KH : (half + 1) * KH, :],
                        xnp_all[:, i, :].unsqueeze(1).to_broadcast((C, KH, C)),
                        mybir.AluOpType.add,
                    )
                    nc.sync.dma_start(
                        out=out_d[i][:, half * HALFN : (half + 1) * HALFN],
                        in_=res_t,
                    )

            loop_cm.__exit__(None, None, None)

    nc.compile()
    return nc


def _get_program():
    global _PROG
    if _PROG is None:
        _PROG = _build_program()
    return _PROG


def _host_inputs(x, Wq, bq, Wk, bk, Wv, bv, Wo, bo, gamma, beta):
    f32 = np.float32
    x = np.asarray(x, f32)
    Wq = np.asarray(Wq, f32)
    bq = np.asarray(bq, f32)
    Wk = np.asarray(Wk, f32)
    bk = np.asarray(bk, f32)
    Wv = np.asarray(Wv, f32)
    bv = np.asarray(bv, f32)
    Wo = np.asarray(Wo, f32)
    bo = np.asarray(bo, f32)
    gamma = np.asarray(gamma, f32)
    beta = np.asarray(beta, f32)

    sc = f32(1.0 / np.sqrt(DK))
    wq97 = np.concatenate(
        [(Wq * sc).reshape(C, NH, DK), (bq * sc).reshape(1, NH, DK)], axis=0
    )
    wk97 = np.concatenate(
        [Wk.reshape(C, NH, DK), bk.reshape(1, NH, DK)], axis=0
    )
    com = {
        "wq": np.ascontiguousarray(wq97),
        "wk": np.ascontiguousarray(wk97),
        "wv": np.ascontiguousarray(Wv.reshape(C, NH, DK)),
        "wo": np.ascontiguousarray(Wo.reshape(NH, DK, C).transpose(1, 0, 2)),
        "gamma": np.ascontiguousarray(gamma.reshape(C, 1)),
        "beta": np.ascontiguousarray(beta.reshape(C, 1)),
        "gmask": np.kron(np.eye(G, dtype=f32), np.ones((C // G, C // G), f32)),
        "ones96": np.ones((S, S), f32),
        "maskT": np.triu(np.ones((S, S), f32)),
        "iden": np.eye(C, dtype=f32),
        "bo_eff": (bv.astype(np.float64) @ Wo.astype(np.float64) + bo)
        .astype(f32)
        .reshape(1, C),
    }
    x_r = np.ascontiguousarray(x.reshape(B_TOTAL, C, C))
    in_maps = []
    for i in range(NCORES):
        m = dict(com)
        m["x"] = np.ascontiguousarray(x_r[i * BPC : (i + 1) * BPC])
        in_maps.append(m)
    return in_maps


def _run(inputs, trace=False):
    from concourse.bass_utils import run_bass_kernel_spmd

    nc = _get_program()
    in_maps = _host_inputs(**inputs)
    res = run_bass_kernel_spmd(
        nc, in_maps, core_ids=list(range(NCORES)), trace=trace
    )
    out = np.concatenate([r["out"] for r in res.results], axis=0)
    return out.reshape(B_TOTAL, C, S, C).astype(np.float32), res


def kernel(**inputs) -> np.ndarray:
    out, _ = _run(inputs, trace=False)
    return out



# revision 36
# speedup vs baseline: 2.4372x; 2.4372x over previous
"""Trainium2 Bass kernel for nn_Attention_43542378447097.

GroupNorm -> multi-head causal self-attention -> out-proj, then the
reference's broadcast add:

    out(B,S,C) + residual(B,C,1,C)  ->  (B,C,S,C)   [right-aligned numpy
    broadcasting, so batches MIX]:

    result[i, j, k, l] = A[j, k, l] + xn[i, j, l]

where A[j] = attention output (incl bo) of batch j and xn[i] = groupnorm
output of batch i.  Output is (96, 96, 96, 96), ~85M elements -> memory
bound on the output write.

v2 strategy (vs the v1 i-sharded fp32 kernel):
  * Shard over j (A's batch index) instead of i: core c owns
    j in J = [12c, 12c+12).  It needs (a) full attention for its own 12
    batches (local), and (b) xn[i, J, :] for ALL 96 batches i -- but the
    groupnorm groups are 8 groups of 12 s-rows, exactly matching the
    shard, so slice (b) needs only group-c stats from x[:, J, :]
    (442 KB).  NO collective at all, and output writes for j-pair t
    start as soon as local batches 2t, 2t+1 finish -> the big output
    DMA overlaps all remaining compute.
  * fp16 everywhere on the wide paths: PE matmuls run 1 cycle/row in
    fp16 vs 4 for fp32; the output is written as fp16 (half the DMA
    bytes; global rel err ~1e-3, well under the 2e-2 gate) and widened
    to fp32 on the host.
  * Assembly layout [l | i, k, j]: partition dim = l is the only dim
    both addends depend on, so both operands are per-partition with
    mid-dim stride-0 broadcasts only (i for the A term, k for the xn
    term) and a contiguous innermost j -> DVE runs in 2x packed mode.
    out[l, i, k, jpair] = Aarr[l, -, k, j] + Xarr[l, i, -, j].
  * Per-core output (6, 96, 96, 96, 2) fp16 [jp, l, i, k, j2], host
    transposes to (i, j, k, l).

Attention per local batch (fp16 PE, fp32 stats):
  xnT (97 rows: xn^T + ones row; q/k biases are a 97th contraction row),
  q/k per head via lhsT=W-slice, v via lhsT=xnT; scoresT = kT_h.T@qT_h
  -> ACT exp -> causal mask mult -> denominators via ones-matmul ->
  reciprocal -> attnT -> oT = v^T@attnT -> A_jT[l,k] = sum_h Wo_h^T@oT_h
  (note lhsT=wo gives A transposed, exactly what assembly needs).
  1/sqrt(dk) folded into Wq/bq, bv folded into bo_eff = bv@Wo + bo,
  bo_eff + beta folded into the Xarr eviction; groupnorm rsqrt is an
  all-DVE Newton iteration so ACT only ever loads the Exp table.
"""

import sys

sys.path.insert(0, "/opt/trn_rl_repo")

import numpy as np

B_TOTAL = 96
C = 96
S = 96
NH = 8
DK = 96
G = 8
NCORES = 8
JPC = B_TOTAL // NCORES  # 12 j's (= local attention batches) per core
NJP = JPC // 2  # 6 j-pairs
EPS = 1e-5

_PROG = None


ASPL = 37  # assembly i-split: DVE rows [0, ASPL), GpSimd rows [ASPL, 48)


def _build_program(loop_n=1, phases="123", skip_collective=True):
    import contextlib

    import concourse.bass as bass
    import concourse.tile as tile
    from concourse import bacc, mybir

    f32 = mybir.dt.float32
    f16 = mybir.dt.float16
    AF = mybir.ActivationFunctionType
    ALU = mybir.AluOpType
    AX = mybir.AxisListType

    nc = bacc.Bacc(
        "TRN2",
        target_bir_lowering=False,
        debug=False,
        enable_asserts=False,
        num_devices=NCORES,
    )

    xall_d = nc.declare_dram_parameter("xall", [S, JPC, C], f16, isOutput=False)
    xg_d = nc.declare_dram_parameter("xg", [B_TOTAL, JPC * C], f16, isOutput=False)
    # all f16 / f32 constants packed into one tensor each: a dozen tiny
    # serialized const DMAs at startup cost ~7 us of dead time otherwise
    cp16_d = nc.declare_dram_parameter("cpack16", [C + 1, 3840], f16, isOutput=False)
    cp32_d = nc.declare_dram_parameter("cpack32", [C, 26], f32, isOutput=False)
    out_d = nc.declare_dram_parameter(
        "out", [NJP, C, B_TOTAL, S, 2], f16, isOutput=True
    )

    with tile.TileContext(nc) as tc:
        with (
            tc.tile_pool(name="const", bufs=1) as cpool,
            tc.tile_pool(name="work", bufs=2) as work,
            tc.tile_pool(name="psum", bufs=6, space="PSUM") as pp,
        ):
            # ---- constants (two packed tiles; views below) ----
            cp16 = cpool.tile([C + 1, 3840], f16, name="cp16")
            cp32 = cpool.tile([C, 26], f32, name="cp32")
            xarr = cpool.tile([C, B_TOTAL, JPC], f16, name="xarr")
            aarr = cpool.tile([C, S, JPC], f16, name="aarr")

            wq_sb = cp16[:, 0:768].rearrange("p (h d) -> p h d", h=NH)
            wk_sb = cp16[:, 768:1536].rearrange("p (h d) -> p h d", h=NH)
            wv_sb = cp16[0:C, 1536:2304].rearrange("p (h d) -> p h d", h=NH)
            wo_sb = cp16[0:DK, 2304:3072].rearrange("p (h d) -> p h d", h=NH)
            gmask_sb = cp16[0:C, 3072:3168]
            ones_sb = cp16[0:S, 3168:3264]
            maskb_sb = cp16[0:S, 3264:3360]
            iden4_sb = cp16[0:C, 3360:3744].rearrange("p (h q) -> p h q", h=4)
            iden_sb = cp16[0:C, 3744:3840]
            gamma_sb = cp32[:, 0:1]
            beta_sb = cp32[:, 1:2]
            gvec_sb = cp32[:, 2:14]
            bb_sb = cp32[:, 14:26]

            nc.sync.dma_start(out=cp16, in_=cp16_d[:])
            nc.sync.dma_start(out=cp32, in_=cp32_d[:])

            inv_na = 1.0 / (C * C // G)  # 1/1152, attention-side groups
            i32 = mybir.dt.int32

            loop_cm = (
                tc.For_i(0, loop_n, 1) if loop_n > 1 else contextlib.nullcontext()
            )
            loop_cm.__enter__()

            def newton_rsqrt(veps, tag, iters=2):
                """rstd = rsqrt(veps), all-DVE (quake seed + Newton steps)
                so ACT only ever needs the Exp table set."""
                shp = list(veps.shape)
                iv = veps.bitcast(i32)
                ineg = work.tile(shp, i32, tag="sti", bufs=8, name="ineg" + tag)
                nc.vector.tensor_scalar_mul(ineg, iv, -1)
                nc.vector.tensor_scalar(ineg, ineg, 1, None, op0=ALU.arith_shift_right)
                nc.vector.tensor_scalar(ineg, ineg, 0x5F3759DF, None, op0=ALU.add)
                y = ineg.bitcast(f32)
                t1 = work.tile(shp, f32, tag="st", bufs=8, name="t1" + tag)
                for _ in range(iters):
                    nc.vector.tensor_mul(t1, y, y)
                    nc.vector.tensor_mul(t1, t1, veps)
                    nc.vector.tensor_scalar(t1, t1, -0.5, 1.5, op0=ALU.mult, op1=ALU.add)
                    nc.vector.tensor_mul(y, y, t1)
                return y

            # ===== prologue 2: groupnorm stats for ALL 12 attention batches
            # (one [C, 12] micro-chain); per-batch xn becomes a single ACT
            # Identity op so the in-loop attention chain never touches DVE
            # except the softmax reciprocal.
            xall_sb = cpool.tile([S, JPC, C], f16, name="xall_sb")
            scale_tb = cpool.tile([C, JPC], f32, name="scale_tb")
            shift_tb = cpool.tile([C, JPC], f32, name="shift_tb")
            if "2" in phases:
                nc.sync.dma_start(out=xall_sb, in_=xall_d[:])
                x2all = work.tile([S, JPC, C], f16, tag="x2all", name="x2all")
                nc.vector.tensor_mul(x2all, xall_sb, xall_sb)
                s12v = work.tile([C, JPC], f32, tag="stv", bufs=8, name="s12v")
                s22v = work.tile([C, JPC], f32, tag="stv", bufs=8, name="s22v")
                for g in range(3):
                    bs = slice(4 * g, 4 * (g + 1))
                    ps1 = pp.tile([C, 4 * C], f32, tag="ps", name="ps_s1")
                    nc.tensor.matmul(
                        ps1,
                        lhsT=gmask_sb,
                        rhs=xall_sb[:, bs, :].rearrange("p b c -> p (b c)"),
                        start=True,
                        stop=True,
                    )
                    nc.vector.tensor_reduce(
                        out=s12v[:, bs].unsqueeze(2),
                        in_=ps1.rearrange("p (b c) -> p b c", b=4),
                        axis=AX.X,
                        op=ALU.add,
                    )
                    ps2 = pp.tile([C, 4 * C], f32, tag="ps", name="ps_s2")
                    nc.tensor.matmul(
                        ps2,
                        lhsT=gmask_sb,
                        rhs=x2all[:, bs, :].rearrange("p b c -> p (b c)"),
                        start=True,
                        stop=True,
                    )
                    nc.vector.tensor_reduce(
                        out=s22v[:, bs].unsqueeze(2),
                        in_=ps2.rearrange("p (b c) -> p b c", b=4),
                        axis=AX.X,
                        op=ALU.add,
                    )
                mu = work.tile([C, JPC], f32, tag="stv", bufs=8, name="mu")
                ex2 = work.tile([C, JPC], f32, tag="stv", bufs=8, name="ex2")
                nc.vector.tensor_scalar_mul(mu, s12v, inv_na)
                nc.vector.tensor_scalar_mul(ex2, s22v, inv_na)
                musq = work.tile([C, JPC], f32, tag="stv", bufs=8, name="musq")
                nc.vector.tensor_mul(musq, mu, mu)
                veps = work.tile([C, JPC], f32, tag="stv", bufs=8, name="veps")
                nc.vector.scalar_tensor_tensor(
                    veps, ex2, EPS, musq, op0=ALU.add, op1=ALU.subtract
                )
                y = newton_rsqrt(veps, "a", iters=1)
                nc.vector.tensor_mul(
                    scale_tb, y, gamma_sb.to_broadcast((C, JPC))
                )
                mus = work.tile([C, JPC], f32, tag="stv", bufs=8, name="mus")
                nc.vector.tensor_mul(mus, mu, scale_tb)
                nc.vector.tensor_sub(
                    shift_tb, beta_sb.to_broadcast((C, JPC)), mus
                )

            # ===== prologue 1: xn slices (group-c rows J, ALL 96 batches) ====
            # xarr[l, i, j] = xn[i, 12c+j, l] * gamma[12c+j] + beta[..] + bo_eff[l]
            # The 12 transpose+evict steps are NOT emitted here; xarr_pair(t)
            # below feeds them into the pair loop just-in-time (asm(t) only
            # reads xarr[:, :, 2t:2t+2]).
            xn_s = cpool.tile([B_TOTAL, JPC, C], f16, name="xn_s")
            if "1" in phases:
                xg_sb = work.tile([B_TOTAL, JPC * C], f16, tag="xg", name="xg_sb")
                nc.sync.dma_start(out=xg_sb, in_=xg_d[:])
                sq = work.tile([B_TOTAL, JPC * C], f16, tag="sq", name="sq")
                nc.vector.tensor_mul(sq, xg_sb, xg_sb)
                s1 = work.tile([C, 1], f32, tag="st", bufs=8, name="s1g")
                s2 = work.tile([C, 1], f32, tag="st", bufs=8, name="s2g")
                nc.vector.tensor_reduce(out=s1, in_=xg_sb, axis=AX.X, op=ALU.add)
                nc.vector.tensor_reduce(out=s2, in_=sq, axis=AX.X, op=ALU.add)
                mu_g = work.tile([C, 1], f32, tag="st", bufs=8, name="mu_g")
                ex2_g = work.tile([C, 1], f32, tag="st", bufs=8, name="ex2_g")
                nc.vector.tensor_scalar_mul(mu_g, s1, inv_na)
                nc.vector.tensor_scalar_mul(ex2_g, s2, inv_na)
                musq_g = work.tile([C, 1], f32, tag="st", bufs=8, name="musq_g")
                nc.vector.tensor_mul(musq_g, mu_g, mu_g)
                veps_g = work.tile([C, 1], f32, tag="st", bufs=8, name="veps_g")
                nc.vector.scalar_tensor_tensor(
                    veps_g, ex2_g, EPS, musq_g, op0=ALU.add, op1=ALU.subtract
                )
                rstd_g = newton_rsqrt(veps_g, "g")
                nc.vector.tensor_scalar(
                    xn_s.rearrange("p j l -> p (j l)"),
                    xg_sb,
                    mu_g,
                    rstd_g,
                    op0=ALU.subtract,
                    op1=ALU.mult,
                )

            def xarr_pair(t):
                if "1" not in phases:
                    return
                for j in (2 * t, 2 * t + 1):
                    ps_t = pp.tile([C, B_TOTAL], f16, tag="ps", name="ps_tj")
                    nc.tensor.transpose(ps_t, xn_s[:, j, :], iden_sb)
                    nc.scalar.activation(
                        out=xarr[:, :, j],
                        in_=ps_t,
                        func=AF.Identity,
                        scale=gvec_sb[:, j : j + 1],
                        bias=bb_sb[:, j : j + 1],
                    )

            # ===== attention for the 12 local batches =====================
            st = {}

            def st1(b):
                d = st[b] = {}
                xn16 = work.tile([S, C], f16, tag="xn16", bufs=4, name="xn16")
                nc.scalar.activation(
                    out=xn16,
                    in_=xall_sb[:, b, :],
                    func=AF.Identity,
                    scale=scale_tb[:, b : b + 1],
                    bias=shift_tb[:, b : b + 1],
                )
                ps_xt = pp.tile([C, S], f16, tag="ps", name="ps_xt")
                nc.tensor.transpose(ps_xt, xn16, iden_sb)
                xnT = work.tile([C + 1, S], f16, tag="xnT", bufs=4, name="xnT")
                nc.scalar.activation(out=xnT[0:C, :], in_=ps_xt, func=AF.Copy)
                nc.vector.memset(xnT[C : C + 1, :], 1.0)
                d["xnT"] = xnT

            def st2(b):
                d = st[b]
                xnT = d["xnT"]
                qT = work.tile([DK, NH, S], f16, tag="qT", bufs=4, name="qT")
                kT = work.tile([DK, NH, S], f16, tag="kT", bufs=4, name="kT")
                v = work.tile([S, NH, DK], f16, tag="v", bufs=4, name="v")
                for hh in range(2):
                    psq = pp.tile([DK, 4 * S], f32, tag="ps", name="ps_q")
                    psk = pp.tile([DK, 4 * S], f32, tag="ps", name="ps_k")
                    psv = pp.tile([S, 4 * DK], f32, tag="ps", name="ps_v")
                    for hl in range(4):
                        h = 4 * hh + hl
                        nc.tensor.matmul(
                            psq[:, hl * S : (hl + 1) * S],
                            lhsT=wq_sb[:, h, :],
                            rhs=xnT,
                            start=True,
                            stop=True,
                        )
                        nc.tensor.matmul(
                            psk[:, hl * S : (hl + 1) * S],
                            lhsT=wk_sb[:, h, :],
                            rhs=xnT,
                            start=True,
                            stop=True,
                        )
                        nc.tensor.matmul(
                            psv[:, hl * DK : (hl + 1) * DK],
                            lhsT=xnT[0:C, :],
                            rhs=wv_sb[:, h, :],
                            start=True,
                            stop=True,
                        )
                    sl = slice(4 * hh, 4 * (hh + 1))
                    nc.scalar.activation(
                        out=qT[:, sl, :].rearrange("p h s -> p (h s)"),
                        in_=psq,
                        func=AF.Copy,
                    )
                    nc.scalar.activation(
                        out=kT[:, sl, :].rearrange("p h s -> p (h s)"),
                        in_=psk,
                        func=AF.Copy,
                    )
                    nc.scalar.activation(
                        out=v[:, sl, :].rearrange("p h s -> p (h s)"),
                        in_=psv,
                        func=AF.Copy,
                    )
                d["qT"], d["kT"], d["v"] = qT, kT, v

            def st3(b):
                # scoresT with the causal mask ADDED in-psum by one extra
                # matmul (lhsT=maskbT, rhs=4x-replicated identity): masked
                # entries get -30, exp flushes them to exactly 0 in f16 --
                # no separate mask multiply on DVE.
                d = st[b]
                expT = work.tile([S, NH, S], f16, tag="expT", bufs=4, name="expT")
                for hh in range(2):
                    pst = pp.tile([S, 4 * S], f32, tag="ps", name="ps_sc")
                    nc.tensor.matmul(
                        pst,
                        lhsT=maskb_sb,
                        rhs=iden4_sb[:].rearrange("c h q -> c (h q)"),
                        start=True,
                        stop=False,
                    )
                    for hl in range(4):
                        h = 4 * hh + hl
                        nc.tensor.matmul(
                            pst[:, hl * S : (hl + 1) * S],
                            lhsT=d["kT"][:, h, :],
                            rhs=d["qT"][:, h, :],
                            start=False,
                            stop=(hl == 3),
                        )
                    sl = slice(4 * hh, 4 * (hh + 1))
                    nc.scalar.activation(
                        out=expT[:, sl, :].rearrange("p h s -> p (h s)"),
                        in_=pst,
                        func=AF.Exp,
                    )
                d["expT"] = expT

            def st4(b):
                d = st[b]
                expT = d["expT"]
                recip = work.tile([S, NH, S], f16, tag="recip", name="recip")
                for hh in range(2):
                    psd = pp.tile([S, 4 * S], f32, tag="ps", name="ps_den")
                    nc.tensor.matmul(
                        psd,
                        lhsT=ones_sb,
                        rhs=expT[:, 4 * hh : 4 * (hh + 1), :].rearrange(
                            "p h s -> p (h s)"
                        ),
                        start=True,
                        stop=True,
                    )
                    with nc.allow_low_precision(
                        reason="attn weights in f16; 2e-2 gate"
                    ):
                        nc.vector.reciprocal(
                            out=recip[:, 4 * hh : 4 * (hh + 1), :].rearrange(
                                "p h s -> p (h s)"
                            ),
                            in_=psd,
                        )
                # normalize in-place on GpSimd (SBUF-only: GPSIMD cannot
                # touch PSUM, the BIR verifier rejects it)
                nc.gpsimd.tensor_tensor(expT, expT, recip, ALU.mult)

            def st5(b):
                d = st.pop(b)
                ocatT = work.tile([DK, NH, S], f16, tag="ocatT", name="ocatT")
                for hh in range(2):
                    pso = pp.tile([DK, 4 * S], f32, tag="ps", name="ps_o")
                    for hl in range(4):
                        h = 4 * hh + hl
                        nc.tensor.matmul(
                            pso[:, hl * S : (hl + 1) * S],
                            lhsT=d["v"][:, h, :],
                            rhs=d["expT"][:, h, :],
                            start=True,
                            stop=True,
                        )
                    sl = slice(4 * hh, 4 * (hh + 1))
                    nc.scalar.activation(
                        out=ocatT[:, sl, :].rearrange("p h s -> p (h s)"),
                        in_=pso,
                        func=AF.Copy,
                    )
                # A_jT[l, k] = sum_h Wo_h^T @ oT_h  (+ bo_eff via bb in xarr)
                psw = pp.tile([C, S], f32, tag="psw", bufs=2, name="ps_w")
                for h in range(NH):
                    nc.tensor.matmul(
                        psw,
                        lhsT=wo_sb[:, h, :],
                        rhs=ocatT[:, h, :],
                        start=(h == 0),
                        stop=(h == NH - 1),
                    )
                nc.scalar.activation(out=aarr[:, :, b], in_=psw, func=AF.Copy)

            # ===== assembly: out[l, i, k, j2] = A + xn, fp16, j-pair t =====
            # Each i-half chunk is split between DVE (rows < ASPL, f16 2x
            # packed mode) and GpSimd (rows >= ASPL) so both engines chew on
            # it concurrently; one DMA per chunk once both parts land.
            def asm_chunk(t, ih):
                if "3" not in phases:
                    return
                res = work.tile(
                    [C, B_TOTAL // 2, S, 2], f16, tag="res", bufs=3, name="res"
                )
                ab = aarr[:, :, 2 * t : 2 * t + 2].unsqueeze(1)
                xb_ = xarr[:, ih * 48 : (ih + 1) * 48, 2 * t : 2 * t + 2].unsqueeze(2)
                nc.vector.tensor_tensor(
                    res[:, 0:ASPL, :, :],
                    ab.to_broadcast((C, ASPL, S, 2)),
                    xb_[:, 0:ASPL].to_broadcast((C, ASPL, S, 2)),
                    ALU.add,
                )
                nc.gpsimd.tensor_tensor(
                    res[:, ASPL:48, :, :],
                    ab.to_broadcast((C, 48 - ASPL, S, 2)),
                    xb_[:, ASPL:48].to_broadcast((C, 48 - ASPL, S, 2)),
                    ALU.add,
                )
                nc.sync.dma_start(
                    out=out_d[t][:, ih * 48 : (ih + 1) * 48, :, :], in_=res
                )

            # Software pipeline: assembly of pair t-1 is emitted at the top
            # of pair t so DVE/GpSimd chew on it while pair t's chain runs
            # on ACT/PE, and the out-DMAs pace one pair behind.
            if "2" in phases:
                for t in range(NJP):
                    xarr_pair(t)
                    if t > 0:
                        asm_chunk(t - 1, 0)
                        asm_chunk(t - 1, 1)
                    st1(2 * t)
                    st1(2 * t + 1)
                    st2(2 * t)
                    st2(2 * t + 1)
                    st3(2 * t)
                    st3(2 * t + 1)
                    st4(2 * t)
                    st4(2 * t + 1)
                    st5(2 * t)
                    st5(2 * t + 1)
                asm_chunk(NJP - 1, 0)
                asm_chunk(NJP - 1, 1)

            loop_cm.__exit__(None, None, None)

    nc.compile()
    return nc


def _get_program():
    global _PROG
    if _PROG is None:
        _PROG = _build_program()
    return _PROG


def _host_inputs(x, Wq, bq, Wk, bk, Wv, bv, Wo, bo, gamma, beta):
    f32 = np.float32
    f16 = np.float16
    x = np.asarray(x, f32)
    Wq = np.asarray(Wq, f32)
    bq = np.asarray(bq, f32)
    Wk = np.asarray(Wk, f32)
    bk = np.asarray(bk, f32)
    Wv = np.asarray(Wv, f32)
    bv = np.asarray(bv, f32)
    Wo = np.asarray(Wo, f32)
    bo = np.asarray(bo, f32)
    gamma = np.asarray(gamma, f32)
    beta = np.asarray(beta, f32)

    sc = f32(1.0 / np.sqrt(DK))
    wq97 = np.concatenate(
        [(Wq * sc).reshape(C, NH, DK), (bq * sc).reshape(1, NH, DK)], axis=0
    )
    wk97 = np.concatenate([Wk.reshape(C, NH, DK), bk.reshape(1, NH, DK)], axis=0)
    bo_eff = (bv.astype(np.float64) @ Wo.astype(np.float64) + bo).astype(f32)

    cp16 = np.zeros((C + 1, 3840), f16)
    cp16[:, 0:768] = wq97.reshape(C + 1, 768).astype(f16)
    cp16[:, 768:1536] = wk97.reshape(C + 1, 768).astype(f16)
    cp16[0:C, 1536:2304] = Wv.reshape(C, 768).astype(f16)
    cp16[0:DK, 2304:3072] = (
        Wo.reshape(NH, DK, C).transpose(1, 0, 2).reshape(DK, 768).astype(f16)
    )
    cp16[0:C, 3072:3168] = np.kron(
        np.eye(G, dtype=f16), np.ones((C // G, C // G), f16)
    )
    cp16[0:S, 3168:3264] = np.ones((S, S), f16)
    # maskbT[q, t] = -30 where t > q (causal), added to scoresT in-psum
    cp16[0:S, 3264:3360] = np.triu(np.full((S, S), -30.0, f16), 1)
    cp16[0:C, 3360:3744] = np.broadcast_to(
        np.eye(C, dtype=f16)[:, None, :], (C, 4, S)
    ).reshape(C, 384)
    cp16[0:C, 3744:3840] = np.eye(C, dtype=f16)

    com = {"cpack16": cp16}
    x_r = np.ascontiguousarray(x.reshape(B_TOTAL, C, C))
    in_maps = []
    for c in range(NCORES):
        J = slice(c * JPC, (c + 1) * JPC)
        m = dict(com)
        # [s, b, c] f16 so the one upfront DMA lands as SBUF [s | (b, c)]
        m["xall"] = np.ascontiguousarray(
            x_r[J].transpose(1, 0, 2).astype(f16)
        )
        m["xg"] = (
            np.ascontiguousarray(x_r[:, J, :])
            .reshape(B_TOTAL, JPC * C)
            .astype(f16)
        )
        cp32 = np.zeros((C, 26), f32)
        cp32[:, 0] = gamma
        cp32[:, 1] = beta
        cp32[:, 2:14] = np.broadcast_to(gamma[J][None, :], (C, JPC))
        cp32[:, 14:26] = beta[J][None, :] + bo_eff[:, None]
        m["cpack32"] = cp32
        in_maps.append(m)
    return in_maps


def _assemble(parts):
    """parts[c]: (NJP, C, B, S, 2) f16 [jp, l, i, k, j2] -> (B, C, S, C) f32."""
    cols = []
    for a in parts:
        a = np.asarray(a).astype(np.float32).reshape(NJP, C, B_TOTAL, S, 2)
        # (jp, l, i, k, j2) -> (i, jp, j2, k, l)
        a = a.transpose(2, 0, 4, 3, 1).reshape(B_TOTAL, JPC, S, C)
        cols.append(a)
    return np.concatenate(cols, axis=1)


def _run(inputs, trace=False):
    from concourse.bass_utils import run_bass_kernel_spmd

    nc = _get_program()
    in_maps = _host_inputs(**inputs)
    res = run_bass_kernel_spmd(
        nc, in_maps, core_ids=list(range(NCORES)), trace=trace
    )
    out = _assemble([r["out"] for r in res.results])
    return out, res


def kernel(**inputs) -> np.ndarray:
    out, _ = _run(inputs, trace=False)
    return out


# revision 43
# speedup vs baseline: 2.9743x; 1.2204x over previous
"""Trainium2 Bass kernel for nn_Attention_43542378447097.

GroupNorm -> multi-head causal self-attention -> out-proj, then the
reference's broadcast add:

    out(B,S,C) + residual(B,C,1,C)  ->  (B,C,S,C)   [right-aligned numpy
    broadcasting, so batches MIX]:

    result[i, j, k, l] = A[j, k, l] + xn[i, j, l]

where A[j] = attention output (incl bo) of batch j and xn[i] = groupnorm
output of batch i.  Output is (96, 96, 96, 96), ~85M elements -> memory
bound on the output write.

v2 strategy (vs the v1 i-sharded fp32 kernel):
  * Shard over j (A's batch index) instead of i: core c owns
    j in J = [12c, 12c+12).  It needs (a) full attention for its own 12
    batches (local), and (b) xn[i, J, :] for ALL 96 batches i -- but the
    groupnorm groups are 8 groups of 12 s-rows, exactly matching the
    shard, so slice (b) needs only group-c stats from x[:, J, :]
    (442 KB).  NO collective at all, and output writes for j-pair t
    start as soon as local batches 2t, 2t+1 finish -> the big output
    DMA overlaps all remaining compute.
  * fp16 everywhere on the wide paths: PE matmuls run 1 cycle/row in
    fp16 vs 4 for fp32; the output is written as fp16 (half the DMA
    bytes; global rel err ~1e-3, well under the 2e-2 gate) and widened
    to fp32 on the host.
  * Assembly layout [l | i, k, j]: partition dim = l is the only dim
    both addends depend on, so both operands are per-partition with
    mid-dim stride-0 broadcasts only (i for the A term, k for the xn
    term) and a contiguous innermost j -> DVE runs in 2x packed mode.
    out[l, i, k, jpair] = Aarr[l, -, k, j] + Xarr[l, i, -, j].
  * Per-core output (6, 96, 96, 96, 2) fp16 [jp, l, i, k, j2], host
    transposes to (i, j, k, l).

Attention per local batch (fp16 PE, fp32 stats):
  xnT (97 rows: xn^T + ones row; q/k biases are a 97th contraction row),
  q/k per head via lhsT=W-slice, v via lhsT=xnT; scoresT = kT_h.T@qT_h
  -> ACT exp -> causal mask mult -> denominators via ones-matmul ->
  reciprocal -> attnT -> oT = v^T@attnT -> A_jT[l,k] = sum_h Wo_h^T@oT_h
  (note lhsT=wo gives A transposed, exactly what assembly needs).
  1/sqrt(dk) folded into Wq/bq, bv folded into bo_eff = bv@Wo + bo,
  bo_eff + beta folded into the Xarr eviction; groupnorm rsqrt is an
  all-DVE Newton iteration so ACT only ever loads the Exp table.
"""

import sys

sys.path.insert(0, "/opt/trn_rl_repo")

import numpy as np

B_TOTAL = 96
C = 96
S = 96
NH = 8
DK = 96
G = 8
NCORES = 8
JPC = B_TOTAL // NCORES  # 12 j's (= local attention batches) per core
NJP = JPC // 2  # 6 j-pairs
EPS = 1e-5

_PROG = None


ASPL = 37  # assembly i-split: DVE rows [0, ASPL), GpSimd rows [ASPL, 48)


def _build_program(loop_n=1, phases="123", skip_collective=True):
    import contextlib

    import concourse.bass as bass
    import concourse.tile as tile
    from concourse import bacc, mybir

    f32 = mybir.dt.float32
    f16 = mybir.dt.float16
    AF = mybir.ActivationFunctionType
    ALU = mybir.AluOpType
    AX = mybir.AxisListType

    nc = bacc.Bacc(
        "TRN2",
        target_bir_lowering=False,
        debug=False,
        enable_asserts=False,
        num_devices=NCORES,
    )

    xall_d = nc.declare_dram_parameter("xall", [S, JPC, C], f16, isOutput=False)
    xg_d = nc.declare_dram_parameter("xg", [B_TOTAL, JPC * C], f16, isOutput=False)
    # all f16 / f32 constants packed into one tensor each: a dozen tiny
    # serialized const DMAs at startup cost ~7 us of dead time otherwise
    cp16_d = nc.declare_dram_parameter("cpack16", [C + 1, 3840], f16, isOutput=False)
    cp32_d = nc.declare_dram_parameter("cpack32", [C, 26], f32, isOutput=False)
    out_d = nc.declare_dram_parameter(
        "out", [NJP, C, B_TOTAL, S, 2], f16, isOutput=True
    )

    with tile.TileContext(nc) as tc:
        with (
            tc.tile_pool(name="const", bufs=1) as cpool,
            tc.tile_pool(name="work", bufs=2) as work,
            tc.tile_pool(name="psum", bufs=6, space="PSUM") as pp,
        ):
            # ---- constants (two packed tiles; views below) ----
            cp16 = cpool.tile([C + 1, 3840], f16, name="cp16")
            cp32 = cpool.tile([C, 26], f32, name="cp32")
            xarr = cpool.tile([C, B_TOTAL, JPC], f16, name="xarr")
            aarr = cpool.tile([C, S, JPC], f16, name="aarr")

            wq_sb = cp16[:, 0:768].rearrange("p (h d) -> p h d", h=NH)
            wk_sb = cp16[:, 768:1536].rearrange("p (h d) -> p h d", h=NH)
            wv_sb = cp16[0:C, 1536:2304].rearrange("p (h d) -> p h d", h=NH)
            wo_sb = cp16[0:DK, 2304:3072].rearrange("p (h d) -> p h d", h=NH)
            gmask_sb = cp16[0:C, 3072:3168]
            ones_sb = cp16[0:S, 3168:3264]
            maskb_sb = cp16[0:S, 3264:3360]
            iden4_sb = cp16[0:C, 3360:3744].rearrange("p (h q) -> p h q", h=4)
            iden_sb = cp16[0:C, 3744:3840]
            gamma_sb = cp32[:, 0:1]
            beta_sb = cp32[:, 1:2]
            gvec_sb = cp32[:, 2:14]
            bb_sb = cp32[:, 14:26]

            nc.sync.dma_start(out=cp16, in_=cp16_d[:])
            nc.sync.dma_start(out=cp32, in_=cp32_d[:])

            inv_na = 1.0 / (C * C // G)  # 1/1152, attention-side groups
            i32 = mybir.dt.int32

            loop_cm = (
                tc.For_i(0, loop_n, 1) if loop_n > 1 else contextlib.nullcontext()
            )
            loop_cm.__enter__()

            def newton_rsqrt(veps, tag, iters=2):
                """rstd = rsqrt(veps), all-DVE (quake seed + Newton steps)
                so ACT only ever needs the Exp table set."""
                shp = list(veps.shape)
                iv = veps.bitcast(i32)
                ineg = work.tile(shp, i32, tag="sti", bufs=8, name="ineg" + tag)
                nc.vector.tensor_scalar_mul(ineg, iv, -1)
                nc.vector.tensor_scalar(ineg, ineg, 1, None, op0=ALU.arith_shift_right)
                nc.vector.tensor_scalar(ineg, ineg, 0x5F3759DF, None, op0=ALU.add)
                y = ineg.bitcast(f32)
                t1 = work.tile(shp, f32, tag="st", bufs=8, name="t1" + tag)
                for _ in range(iters):
                    nc.vector.tensor_mul(t1, y, y)
                    nc.vector.tensor_mul(t1, t1, veps)
                    nc.vector.tensor_scalar(t1, t1, -0.5, 1.5, op0=ALU.mult, op1=ALU.add)
                    nc.vector.tensor_mul(y, y, t1)
                return y

            # ===== prologue 2: groupnorm stats for ALL 12 attention batches
            # (one [C, 12] micro-chain); per-batch xn becomes a single ACT
            # Identity op so the in-loop attention chain never touches DVE
            # except the softmax reciprocal.
            xall_sb = cpool.tile([S, JPC, C], f16, name="xall_sb")
            scale_tb = cpool.tile([C, JPC], f32, name="scale_tb")
            shift_tb = cpool.tile([C, JPC], f32, name="shift_tb")
            if "2" in phases:
                nc.sync.dma_start(out=xall_sb, in_=xall_d[:])
                x2all = work.tile([S, JPC, C], f16, tag="x2all", name="x2all")
                nc.vector.tensor_mul(x2all, xall_sb, xall_sb)
                s12v = work.tile([C, JPC], f32, tag="stv", bufs=8, name="s12v")
                s22v = work.tile([C, JPC], f32, tag="stv", bufs=8, name="s22v")
                for g in range(3):
                    bs = slice(4 * g, 4 * (g + 1))
                    ps1 = pp.tile([C, 4 * C], f32, tag="pss", bufs=2, name="ps_s1")
                    nc.tensor.matmul(
                        ps1,
                        lhsT=gmask_sb,
                        rhs=xall_sb[:, bs, :].rearrange("p b c -> p (b c)"),
                        start=True,
                        stop=True,
                    )
                    nc.vector.tensor_reduce(
                        out=s12v[:, bs].unsqueeze(2),
                        in_=ps1.rearrange("p (b c) -> p b c", b=4),
                        axis=AX.X,
                        op=ALU.add,
                    )
                    ps2 = pp.tile([C, 4 * C], f32, tag="pss", bufs=2, name="ps_s2")
                    nc.tensor.matmul(
                        ps2,
                        lhsT=gmask_sb,
                        rhs=x2all[:, bs, :].rearrange("p b c -> p (b c)"),
                        start=True,
                        stop=True,
                    )
                    nc.vector.tensor_reduce(
                        out=s22v[:, bs].unsqueeze(2),
                        in_=ps2.rearrange("p (b c) -> p b c", b=4),
                        axis=AX.X,
                        op=ALU.add,
                    )
                mu = work.tile([C, JPC], f32, tag="stv", bufs=8, name="mu")
                ex2 = work.tile([C, JPC], f32, tag="stv", bufs=8, name="ex2")
                nc.vector.tensor_scalar_mul(mu, s12v, inv_na)
                nc.vector.tensor_scalar_mul(ex2, s22v, inv_na)
                musq = work.tile([C, JPC], f32, tag="stv", bufs=8, name="musq")
                nc.vector.tensor_mul(musq, mu, mu)
                veps = work.tile([C, JPC], f32, tag="stv", bufs=8, name="veps")
                nc.vector.scalar_tensor_tensor(
                    veps, ex2, EPS, musq, op0=ALU.add, op1=ALU.subtract
                )
                y = newton_rsqrt(veps, "a", iters=1)
                nc.vector.tensor_mul(
                    scale_tb, y, gamma_sb.to_broadcast((C, JPC))
                )
                mus = work.tile([C, JPC], f32, tag="stv", bufs=8, name="mus")
                nc.vector.tensor_mul(mus, mu, scale_tb)
                nc.vector.tensor_sub(
                    shift_tb, beta_sb.to_broadcast((C, JPC)), mus
                )

            # ===== prologue 1: xn slices (group-c rows J, ALL 96 batches) ====
            # xarr[l, i, j] = xn[i, 12c+j, l] * gamma[12c+j] + beta[..] + bo_eff[l]
            # The 12 transpose+evict steps are NOT emitted here; xarr_pair(t)
            # below feeds them into the pair loop just-in-time (asm(t) only
            # reads xarr[:, :, 2t:2t+2]).
            xn_s = cpool.tile([B_TOTAL, JPC, C], f16, name="xn_s")
            if "1" in phases:
                xg_sb = work.tile([B_TOTAL, JPC * C], f16, tag="xg", name="xg_sb")
                nc.sync.dma_start(out=xg_sb, in_=xg_d[:])
                sq = work.tile([B_TOTAL, JPC * C], f16, tag="sq", name="sq")
                nc.vector.tensor_mul(sq, xg_sb, xg_sb)
                s1 = work.tile([C, 1], f32, tag="st", bufs=8, name="s1g")
                s2 = work.tile([C, 1], f32, tag="st", bufs=8, name="s2g")
                nc.vector.tensor_reduce(out=s1, in_=xg_sb, axis=AX.X, op=ALU.add)
                nc.vector.tensor_reduce(out=s2, in_=sq, axis=AX.X, op=ALU.add)
                mu_g = work.tile([C, 1], f32, tag="st", bufs=8, name="mu_g")
                ex2_g = work.tile([C, 1], f32, tag="st", bufs=8, name="ex2_g")
                nc.vector.tensor_scalar_mul(mu_g, s1, inv_na)
                nc.vector.tensor_scalar_mul(ex2_g, s2, inv_na)
                musq_g = work.tile([C, 1], f32, tag="st", bufs=8, name="musq_g")
                nc.vector.tensor_mul(musq_g, mu_g, mu_g)
                veps_g = work.tile([C, 1], f32, tag="st", bufs=8, name="veps_g")
                nc.vector.scalar_tensor_tensor(
                    veps_g, ex2_g, EPS, musq_g, op0=ALU.add, op1=ALU.subtract
                )
                rstd_g = newton_rsqrt(veps_g, "g")
                nc.vector.tensor_scalar(
                    xn_s.rearrange("p j l -> p (j l)"),
                    xg_sb,
                    mu_g,
                    rstd_g,
                    op0=ALU.subtract,
                    op1=ALU.mult,
                )

            def xarr_pair(t):
                if "1" not in phases:
                    return
                for j in (2 * t, 2 * t + 1):
                    ps_t = pp.tile([C, B_TOTAL], f16, tag="pss", bufs=2, name="ps_tj")
                    nc.tensor.transpose(ps_t, xn_s[:, j, :], iden_sb)
                    nc.scalar.activation(
                        out=xarr[:, :, j],
                        in_=ps_t,
                        func=AF.Identity,
                        scale=gvec_sb[:, j : j + 1],
                        bias=bb_sb[:, j : j + 1],
                    )

            # calibration variant ("3" without "2"): zero aarr so the
            # assembly chunks are runnable without the attention stages
            if "2" not in phases and "3" in phases:
                nc.vector.memset(aarr[:], 0.0)

            # ===== attention for the 12 local batches =====================
            st = {}

            def st1(b):
                d = st[b] = {}
                xn16 = work.tile([S, C], f16, tag="xn16", bufs=4, name="xn16")
                nc.scalar.activation(
                    out=xn16,
                    in_=xall_sb[:, b, :],
                    func=AF.Identity,
                    scale=scale_tb[:, b : b + 1],
                    bias=shift_tb[:, b : b + 1],
                )
                ps_xt = pp.tile([C, S], f16, tag="pss", bufs=2, name="ps_xt")
                nc.tensor.transpose(ps_xt, xn16, iden_sb)
                xnT = work.tile([C + 1, S], f16, tag="xnT", bufs=4, name="xnT")
                nc.scalar.activation(out=xnT[0:C, :], in_=ps_xt, func=AF.Copy)
                nc.vector.memset(xnT[C : C + 1, :], 1.0)
                d["xnT"] = xnT

            def st2(b):
                d = st[b]
                xnT = d["xnT"]
                # q and k land in one 2-bank psum pair tile per head-group
                # (q in bank 0, k in bank 1) so each eviction is a single
                # ACT op covering both; v pairs its two head-groups the
                # same way.  Halves the ACT op count of the old layout.
                qkT = work.tile([DK, 2, NH, S], f16, tag="qkT", bufs=4, name="qkT")
                v = work.tile([S, NH, DK], f16, tag="v", bufs=4, name="v")
                for hh in range(2):
                    psqk = pp.tile([DK, 1024], f32, tag="ps", bufs=2, name="ps_qk")
                    for hl in range(4):
                        h = 4 * hh + hl
                        nc.tensor.matmul(
                            psqk[:, hl * S : (hl + 1) * S],
                            lhsT=wq_sb[:, h, :],
                            rhs=xnT,
                            start=True,
                            stop=True,
                        )
                        nc.tensor.matmul(
                            psqk[:, 512 + hl * S : 512 + (hl + 1) * S],
                            lhsT=wk_sb[:, h, :],
                            rhs=xnT,
                            start=True,
                            stop=True,
                        )
                    sl = slice(4 * hh, 4 * (hh + 1))
                    nc.scalar.activation(
                        out=qkT[:, :, sl, :],
                        in_=psqk.rearrange("p (q x) -> p q x", q=2)[
                            :, :, 0:384
                        ].rearrange("p q (h s) -> p q h s", h=4),
                        func=AF.Copy,
                    )
                psv = pp.tile([S, 1024], f32, tag="ps", bufs=2, name="ps_v")
                for hh in range(2):
                    for hl in range(4):
                        h = 4 * hh + hl
                        nc.tensor.matmul(
                            psv[:, 512 * hh + hl * DK : 512 * hh + (hl + 1) * DK],
                            lhsT=xnT[0:C, :],
                            rhs=wv_sb[:, h, :],
                            start=True,
                            stop=True,
                        )
                nc.scalar.activation(
                    out=v.rearrange("p (q h) s -> p q h s", q=2),
                    in_=psv.rearrange("p (q x) -> p q x", q=2)[
                        :, :, 0:384
                    ].rearrange("p q (h s) -> p q h s", h=4),
                    func=AF.Copy,
                )
                d["qkT"], d["v"] = qkT, v

            def st3(b):
                # scoresT with the causal mask ADDED in-psum: the mask matmul
                # opens each bank's accumulation group (start=True over the
                # whole 384-col region), the per-head score matmuls then
                # accumulate into their 96-col slices.  exp of both
                # head-groups is a single ACT op over the 2-bank pair.
                d = st[b]
                qkT = d["qkT"]
                expT = work.tile([S, NH, S], f16, tag="expT", bufs=4, name="expT")
                pst = pp.tile([S, 1024], f32, tag="ps", bufs=2, name="ps_sc")
                for hh in range(2):
                    off = 512 * hh
                    nc.tensor.matmul(
                        pst[:, off : off + 384],
                        lhsT=maskb_sb,
                        rhs=iden4_sb[:].rearrange("c h q -> c (h q)"),
                        start=True,
                        stop=False,
                    )
                    for hl in range(4):
                        h = 4 * hh + hl
                        nc.tensor.matmul(
                            pst[:, off + hl * S : off + (hl + 1) * S],
                            lhsT=qkT[:, 1, h, :],
                            rhs=qkT[:, 0, h, :],
                            start=False,
                            stop=(hl == 3),
                        )
                nc.scalar.activation(
                    out=expT.rearrange("p (q h) s -> p q h s", q=2),
                    in_=pst.rearrange("p (q x) -> p q x", q=2)[
                        :, :, 0:384
                    ].rearrange("p q (h s) -> p q h s", h=4),
                    func=AF.Exp,
                )
                d["expT"] = expT

            def st4(b):
                d = st[b]
                expT = d["expT"]
                recip = work.tile([S, NH, S], f16, tag="recip", name="recip")
                psd = pp.tile([S, 1024], f32, tag="ps", bufs=2, name="ps_den")
                for hh in range(2):
                    nc.tensor.matmul(
                        psd[:, 512 * hh : 512 * hh + 384],
                        lhsT=ones_sb,
                        rhs=expT[:, 4 * hh : 4 * (hh + 1), :].rearrange(
                            "p h s -> p (h s)"
                        ),
                        start=True,
                        stop=True,
                    )
                with nc.allow_low_precision(
                    reason="attn weights in f16; 2e-2 gate"
                ):
                    nc.vector.reciprocal(
                        out=recip.rearrange("p (q h) s -> p q h s", q=2),
                        in_=psd.rearrange("p (q x) -> p q x", q=2)[
                            :, :, 0:384
                        ].rearrange("p q (h s) -> p q h s", h=4),
                    )
                # normalize in-place on GpSimd (SBUF-only: GPSIMD cannot
                # touch PSUM, the BIR verifier rejects it)
                nc.gpsimd.tensor_tensor(expT, expT, recip, ALU.mult)

            def st5(b):
                d = st.pop(b)
                ocatT = work.tile([DK, NH, S], f16, tag="ocatT", name="ocatT")
                pso = pp.tile([DK, 1024], f32, tag="ps", bufs=2, name="ps_o")
                for hh in range(2):
                    for hl in range(4):
                        h = 4 * hh + hl
                        nc.tensor.matmul(
                            pso[:, 512 * hh + hl * S : 512 * hh + (hl + 1) * S],
                            lhsT=d["v"][:, h, :],
                            rhs=d["expT"][:, h, :],
                            start=True,
                            stop=True,
                        )
                nc.scalar.activation(
                    out=ocatT.rearrange("p (q h) s -> p q h s", q=2),
                    in_=pso.rearrange("p (q x) -> p q x", q=2)[
                        :, :, 0:384
                    ].rearrange("p q (h s) -> p q h s", h=4),
                    func=AF.Copy,
                )
                # A_jT[l, k] = sum_h Wo_h^T @ oT_h  (+ bo_eff via bb in xarr)
                psw = pp.tile([C, S], f32, tag="psw", bufs=2, name="ps_w")
                for h in range(NH):
                    nc.tensor.matmul(
                        psw,
                        lhsT=wo_sb[:, h, :],
                        rhs=ocatT[:, h, :],
                        start=(h == 0),
                        stop=(h == NH - 1),
                    )
                nc.scalar.activation(out=aarr[:, :, b], in_=psw, func=AF.Copy)

            # ===== assembly: out[l, i, k, j2] = A + xn, fp16, j-pair t =====
            # Each i-half chunk is split between DVE (rows < ASPL, f16 2x
            # packed mode) and GpSimd (rows >= ASPL) so both engines chew on
            # it concurrently; one DMA per chunk once both parts land.
            def asm_chunk(t, ih):
                if "3" not in phases:
                    return
                res = work.tile(
                    [C, B_TOTAL // 2, S, 2], f16, tag="res", bufs=3, name="res"
                )
                ab = aarr[:, :, 2 * t : 2 * t + 2].unsqueeze(1)
                xb_ = xarr[:, ih * 48 : (ih + 1) * 48, 2 * t : 2 * t + 2].unsqueeze(2)
                nc.vector.tensor_tensor(
                    res[:, 0:ASPL, :, :],
                    ab.to_broadcast((C, ASPL, S, 2)),
                    xb_[:, 0:ASPL].to_broadcast((C, ASPL, S, 2)),
                    ALU.add,
                )
                nc.gpsimd.tensor_tensor(
                    res[:, ASPL:48, :, :],
                    ab.to_broadcast((C, 48 - ASPL, S, 2)),
                    xb_[:, ASPL:48].to_broadcast((C, 48 - ASPL, S, 2)),
                    ALU.add,
                )
                nc.sync.dma_start(
                    out=out_d[t][:, ih * 48 : (ih + 1) * 48, :, :], in_=res
                )

            # Software pipeline: assembly of pair t-1 is emitted at the top
            # of pair t so DVE/GpSimd chew on it while pair t's chain runs
            # on ACT/PE, and the out-DMAs pace one pair behind.
            if "2" not in phases and "3" in phases:
                for t in range(NJP):
                    xarr_pair(t)
                    asm_chunk(t, 0)
                    asm_chunk(t, 1)
            if "2" in phases:
                # groups of 4 batches, stage-interleaved; assembly of the
                # previous group's two pairs at the top of each group
                for g in range(NJP // 2 - 1):
                    xarr_pair(2 * g)
                    xarr_pair(2 * g + 1)
                    if g > 0:
                        for tp in (2 * g - 2, 2 * g - 1):
                            asm_chunk(tp, 0)
                            asm_chunk(tp, 1)
                    bs = [4 * g + i for i in range(4)]
                    for fn in (st1, st2, st3, st4, st5):
                        for b in bs:
                            fn(b)
                # last group pair-wise so asm of its first pair overlaps the
                # second pair's tail; only asm(NJP-1) is fully exposed
                xarr_pair(NJP - 2)
                xarr_pair(NJP - 1)
                for tp in (NJP - 4, NJP - 3):
                    asm_chunk(tp, 0)
                    asm_chunk(tp, 1)
                b0 = 2 * (NJP - 2)
                for fn in (st1, st2, st3, st4, st5):
                    fn(b0)
                    fn(b0 + 1)
                b1 = 2 * (NJP - 1)
                st1(b1)
                st1(b1 + 1)
                st2(b1)
                st2(b1 + 1)
                asm_chunk(NJP - 2, 0)
                asm_chunk(NJP - 2, 1)
                st3(b1)
                st3(b1 + 1)
                st4(b1)
                st4(b1 + 1)
                st5(b1)
                st5(b1 + 1)
                asm_chunk(NJP - 1, 0)
                asm_chunk(NJP - 1, 1)

            loop_cm.__exit__(None, None, None)

    nc.compile()
    return nc


def _get_program():
    global _PROG
    if _PROG is None:
        _PROG = _build_program()
    return _PROG


def _host_inputs(x, Wq, bq, Wk, bk, Wv, bv, Wo, bo, gamma, beta):
    f32 = np.float32
    f16 = np.float16
    x = np.asarray(x, f32)
    Wq = np.asarray(Wq, f32)
    bq = np.asarray(bq, f32)
    Wk = np.asarray(Wk, f32)
    bk = np.asarray(bk, f32)
    Wv = np.asarray(Wv, f32)
    bv = np.asarray(bv, f32)
    Wo = np.asarray(Wo, f32)
    bo = np.asarray(bo, f32)
    gamma = np.asarray(gamma, f32)
    beta = np.asarray(beta, f32)

    sc = f32(1.0 / np.sqrt(DK))
    wq97 = np.concatenate(
        [(Wq * sc).reshape(C, NH, DK), (bq * sc).reshape(1, NH, DK)], axis=0
    )
    wk97 = np.concatenate([Wk.reshape(C, NH, DK), bk.reshape(1, NH, DK)], axis=0)
    bo_eff = (bv.astype(np.float64) @ Wo.astype(np.float64) + bo).astype(f32)

    cp16 = np.zeros((C + 1, 3840), f16)
    cp16[:, 0:768] = wq97.reshape(C + 1, 768).astype(f16)
    cp16[:, 768:1536] = wk97.reshape(C + 1, 768).astype(f16)
    cp16[0:C, 1536:2304] = Wv.reshape(C, 768).astype(f16)
    cp16[0:DK, 2304:3072] = (
        Wo.reshape(NH, DK, C).transpose(1, 0, 2).reshape(DK, 768).astype(f16)
    )
    cp16[0:C, 3072:3168] = np.kron(
        np.eye(G, dtype=f16), np.ones((C // G, C // G), f16)
    )
    cp16[0:S, 3168:3264] = np.ones((S, S), f16)
    # maskbT[q, t] = -30 where t > q (causal), added to scoresT in-psum
    cp16[0:S, 3264:3360] = np.triu(np.full((S, S), -30.0, f16), 1)
    cp16[0:C, 3360:3744] = np.broadcast_to(
        np.eye(C, dtype=f16)[:, None, :], (C, 4, S)
    ).reshape(C, 384)
    cp16[0:C, 3744:3840] = np.eye(C, dtype=f16)

    com = {"cpack16": cp16}
    x_r = np.ascontiguousarray(x.reshape(B_TOTAL, C, C))
    in_maps = []
    for c in range(NCORES):
        J = slice(c * JPC, (c + 1) * JPC)
        m = dict(com)
        # [s, b, c] f16 so the one upfront DMA lands as SBUF [s | (b, c)]
        m["xall"] = np.ascontiguousarray(
            x_r[J].transpose(1, 0, 2).astype(f16)
        )
        m["xg"] = (
            np.ascontiguousarray(x_r[:, J, :])
            .reshape(B_TOTAL, JPC * C)
            .astype(f16)
        )
        cp32 = np.zeros((C, 26), f32)
        cp32[:, 0] = gamma
        cp32[:, 1] = beta
        cp32[:, 2:14] = np.broadcast_to(gamma[J][None, :], (C, JPC))
        cp32[:, 14:26] = beta[J][None, :] + bo_eff[:, None]
        m["cpack32"] = cp32
        in_maps.append(m)
    return in_maps


def _assemble(parts):
    """parts[c]: (NJP, C, B, S, 2) f16 [jp, l, i, k, j2] -> (B, C, S, C) f32."""
    cols = []
    for a in parts:
        a = np.asarray(a).astype(np.float32).reshape(NJP, C, B_TOTAL, S, 2)
        # (jp, l, i, k, j2) -> (i, jp, j2, k, l)
        a = a.transpose(2, 0, 4, 3, 1).reshape(B_TOTAL, JPC, S, C)
        cols.append(a)
    return np.concatenate(cols, axis=1)


def _run(inputs, trace=False):
    from concourse.bass_utils import run_bass_kernel_spmd

    nc = _get_program()
    in_maps = _host_inputs(**inputs)
    res = run_bass_kernel_spmd(
        nc, in_maps, core_ids=list(range(NCORES)), trace=trace
    )
    out = _assemble([r["out"] for r in res.results])
    return out, res


def kernel(**inputs) -> np.ndarray:
    out, _ = _run(inputs, trace=False)
    return out
